# revision 2
# baseline (speedup 1.0000x reference)
"""Causal self-attention (dense transformer) on 8 TRN2 NeuronCores.

Sharding: heads+batch tensor-parallel; each core owns 2 heads for all batches.
Key speedups over the bf16 baseline:
  - QKV projection in compensated fp8 (hi/lo e4m3 splits of x and 32*w prepared
    on the host) with DoubleRow matmuls: 3 passes (hh, hl, lh) at 0.5 cyc/col
    = 0.75x the bf16 cost at bf16-level accuracy.
  - QK scores in fp8 DoubleRow with a zero-padded pair tile: q/k live in one
    SBUF block [q | zeros | k]; the DR pair dim points the second reduction
    tile at the shared zero block, so the 64-deep contraction runs at
    0.5 cyc/col without any partition remapping.
  - Softmax as p = 2^(s') with the 1/sqrt(hd)*log2(e) scale folded into the
    q copy; exp runs on ACT (scale=ln2), diagonal tri-masks on Pool.
  - PV (flipped, ones-column denominator) and output projection stay bf16.
  - Tail: per-batch AllToAll; last unit's staging DMAs go on the ACT queue so
    they are not stuck behind proj out-DMAs on SP; batches 0-2's output
    projections are deferred to fill the last collective's window.

Storage fp32 in/out; PSUM accumulation fp32 everywhere.
"""

import numpy as np
import ml_dtypes

import concourse.bass as bass
import concourse.mybir as mybir
import concourse.tile as tile
from concourse.bass_utils import run_bass_kernel_spmd

BF16 = mybir.dt.bfloat16
F32 = mybir.dt.float32
E4 = mybir.dt.float8e4
AF = mybir.ActivationFunctionType
DR = mybir.MatmulPerfMode.DoubleRow
E4NP = ml_dtypes.float8_e4m3
BF16NP = ml_dtypes.bfloat16

N_CORES = 8
BSZ, SEQ, D, N_HEAD = 4, 2048, 1024, 16
HD = 64

SM_SCALE = float(1.0 / np.sqrt(HD))  # softmax scale, applied inside ACT Exp


def _split_multi_waits(nc):
    """walrus accepts at most ONE sync-wait per instruction: hoist extras onto
    same-engine nops placed immediately before (queue order preserves sems)."""
    edits = []
    for func in nc.m.functions:
        for bb in func.blocks:
            for idx, ins in enumerate(bb.instructions):
                si = ins.sync_info
                if si is not None and len(si.on_wait) > 1:
                    edits.append((bb, idx, ins))
    for bb, idx, ins in reversed(edits):
        si = ins.sync_info
        extra, keep = list(si.on_wait[:-1]), [si.on_wait[-1]]
        ins.sync_info = mybir.SyncInfo(on_wait=keep, on_update=list(si.on_update))
        nops = []
        for w in extra:
            nop = nc.engines[ins.engine].nop().ins
            host = nc.cur_bb.bb.instructions
            assert host[-1] is nop
            host.pop()
            nop.sync_info = mybir.SyncInfo(on_wait=[w], on_update=[])
            nops.append(nop)
        live = bb.instructions
        for j, nop in enumerate(nops):
            live.insert(idx + j, nop)


def build_nc(n_cores=N_CORES, bsz=BSZ, seq=SEQ, d=D, n_head=N_HEAD):
    hd = HD
    hpc = n_head // n_cores          # heads per core
    fl = hpc * hd                    # local feature width
    T = bsz * seq                    # total tokens
    kd = d // 128                    # 128-contraction tiles over d
    npair = kd // 2                  # DoubleRow pairs over d
    tb = 512                         # tq block width
    nqb = seq // tb                  # q-blocks per batch
    dtiles = tb // 128               # 128-token subtiles per q-block
    nt = T // 128                    # total 128-token tiles
    tsb = seq // n_cores             # per-batch token chunk per core (a2a)

    nc = bass.Bass(num_devices=n_cores)
    # x hi/lo fp8 interleaved: tile j = 2*i + hl holds rows i*128..(i+1)*128 of
    # (x hi if hl==0 else x residual)
    xhl = nc.declare_dram_parameter("xhl", [2 * kd, 128, T], E4, isOutput=False)
    # w hi/lo DR layout: [128, jp, g, hl, 3*fl]; contraction row (jp*2+g)*128+p
    whl = nc.declare_dram_parameter("whl", [128, npair, 2, 2, 3 * fl], E4,
                                    isOutput=False)
    wproj = nc.declare_dram_parameter("wproj", [d, d], BF16, isOutput=False)
    out = nc.declare_dram_parameter("out", [d, bsz * tsb], F32, isOutput=True)
    a2a_in = [nc.dram_tensor(f"a2a_in{b}", [n_cores, tsb, fl], BF16)
              for b in range(bsz)]
    a2a_out = [nc.dram_tensor(f"a2a_out{b}", [n_cores, tsb, fl], BF16)
               for b in range(bsz)]

    with tile.TileContext(nc) as tc:
        with (
            tc.tile_pool(name="const", bufs=1) as const,
            tc.tile_pool(name="xin", bufs=2) as xin,
            tc.tile_pool(name="work", bufs=3) as work,
            tc.tile_pool(name="psum", bufs=1, space="PSUM") as psum,
        ):
            # ---- persistent SBUF ----
            w_sb = const.tile([128, npair * 2 * 2 * 3 * fl], E4, name="w_sb")
            wv = w_sb.rearrange("p (j g h c) -> p j g h c", j=npair, g=2, h=2)
            wflat = whl[:].rearrange("p a b c d -> p (a b c d)")
            wseg = 2 * 2 * 3 * fl  # one j-pair's bytes
            nc.scalar.dma_start(w_sb[:, 0:wseg], wflat[:, 0:wseg])
            nc.scalar.dma_start(w_sb[:, wseg:], wflat[:, wseg:])
            wp_sb = const.tile([128, n_cores * d], BF16, name="wp_sb")

            # q | zeros | k fp8 blocks (zero block shared by both DR pairs)
            qkz = const.tile([128, 3 * T], E4, name="qkz")
            nc.gpsimd.memset(qkz[:, T:2 * T], 0.0)

            y_loc = [const.tile([128, n_cores * tsb], BF16, name=f"y_loc{b}")
                     for b in range(bsz)]

            # v token-major bf16 with ones column per (tile, head)
            vw = hpc * (hd + 1)
            v_sb = const.tile([128, nt * vw], BF16, name="v_sb")
            ones_ap = v_sb.rearrange("p (n h c) -> p n h c", h=hpc,
                                     c=hd + 1)[:, :, :, hd:hd + 1]
            nc.vector.memset(ones_ap, 1.0)

            # triangular mask
            tri = const.tile([128, 128], BF16, name="tri")
            nc.gpsimd.memset(tri[:], 1.0)
            nc.gpsimd.affine_select(
                out=tri[:], in_=tri[:],
                compare_op=mybir.AluOpType.is_ge, fill=0.0,
                base=0, channel_multiplier=-1, pattern=[[1, 128]],
            )

            pending = []       # qkv chunks for upcoming units
            pending_proj = []  # output-projection chunks (tail filler)

            def drain(n=None):
                k = len(pending) if n is None else min(n, len(pending))
                for _ in range(k):
                    pending.pop(0)()

            def issue_x_dma(b, qb, split_x=False):
                tbi = b * nqb + qb
                ts0 = tbi * tb
                # [128, 2kd tiles, tb] fp8: tile j at cols j*tb
                x_t = xin.tile([128, 2 * kd * tb], E4, name="x_t", tag="x",
                               bufs=4)
                xsrc = xhl[:, :, ts0:ts0 + tb]
                xdst = x_t[:].rearrange("p (j t) -> p j t", j=2 * kd)
                if split_x:
                    q = (2 * kd) // 4
                    for j in range(4):
                        nc.sync.dma_start(
                            xdst[:, j * q:(j + 1) * q],
                            xsrc[j * q:(j + 1) * q].rearrange("j p t -> p j t"))
                else:
                    nc.sync.dma_start(
                        xdst, xsrc.rearrange("j p t -> p j t"))
                return x_t

            def push_qkv_chunks(b, qb, x_t):
                tbi = b * nqb + qb
                ts0 = tbi * tb
                xv = x_t[:].rearrange("p (j t) -> p j t", j=2 * kd)

                def moving(m, hl, c0, cw):
                    # tiles j = 4m+hl and 4m+2+hl -> [128, 2, cw]
                    ap = xv[:, 4 * m + hl:4 * m + hl + 3:2, c0:c0 + cw]
                    return ap

                def stat(m, hl, c0, cw):
                    # stationary w [128, 2, cw] for pair m, hi/lo hl
                    return wv[:, m, :, hl, c0:c0 + cw]

                # q and k (feature-major [fl, tb]): 3 comp passes x npair DR
                PASSES = ((0, 0), (0, 1), (1, 0))  # (x_hl, w_hl)
                for which, dst0 in ((0, 0), (1, 2 * T)):
                    st = {}

                    def qk_pass(pi, which=which, dst0=dst0, st=st):
                        xh_, wh_ = PASSES[pi]
                        if pi == 0:
                            st["ps"] = psum.tile([fl, tb], F32, name="ps_qk",
                                                 tag="mm512", bufs=2)
                        ps = st["ps"]
                        for m in range(npair):
                            nc.tensor.matmul(
                                ps[:], stat(m, wh_, which * fl, fl),
                                moving(m, xh_, 0, tb),
                                start=(pi == 0 and m == 0),
                                stop=(pi == 2 and m == npair - 1),
                                perf_mode=DR)
                        if pi == 2:
                            # 1/32 undoes the host-side w*32; quantize q/k to
                            # fp8 at natural scale (~N(0,1)) so values stay
                            # clear of e4m3's coarse subnormal range
                            nc.vector.tensor_scalar_mul(
                                qkz[:, dst0 + ts0:dst0 + ts0 + tb], ps[:],
                                1.0 / 32.0)

                    for pi in range(3):
                        pending.append(lambda pi=pi, f=qk_pass: f(pi))

                # v (token-major [128 tok, fl] bf16, scaled 1/32)
                for tt in range(dtiles):
                    vst = {}

                    def v_chunk(tt=tt, vst=vst):
                        gti = tbi * dtiles + tt
                        ps_v = psum.tile([128, fl], F32, name="ps_v",
                                         tag="mm512", bufs=2)
                        for pi in range(3):
                            xh_, wh_ = PASSES[pi]
                            for m in range(npair):
                                nc.tensor.matmul(
                                    ps_v[:],
                                    moving(m, xh_, tt * 128, 128),
                                    stat(m, wh_, 2 * fl, fl),
                                    start=(pi == 0 and m == 0),
                                    stop=(pi == 2 and m == npair - 1),
                                    perf_mode=DR)
                        nc.vector.tensor_scalar_mul(
                            v_sb.rearrange("p (n h c) -> p n h c", h=hpc,
                                           c=hd + 1)[:, gti, :, 0:hd],
                            ps_v[:].rearrange("p (h c) -> p h c", c=hd),
                            1.0 / 32.0)

                    pending.append(v_chunk)

            qz = qkz[:].rearrange("p (z t) -> p z t", z=3)

            def qk_scores(b, qb, tki):
                tq0 = b * seq + qb * tb
                t0k = b * seq + tki * 128
                m = tki - qb * dtiles
                c0 = 128 * m if m > 0 else 0
                ps_s = psum.tile([128, hpc * tb], F32, name="ps_s",
                                 tag="s2", bufs=2)
                p_t = work.tile([128, hpc * tb], BF16, name="p_t",
                                tag="pt", bufs=6)
                for h in range(hpc):
                    hs = slice(h * hd, (h + 1) * hd)
                    # lhsT: [64, 2, 128]: pair g0 = k block (z=2), g1 = zeros
                    lhsT = qz[hs, 1:3, t0k:t0k + 128][:, ::-1]
                    rhs = qz[hs, 0:2, tq0 + c0:tq0 + tb]
                    nc.tensor.matmul(ps_s[:, h * tb + c0:(h + 1) * tb],
                                     lhsT, rhs, start=True, stop=True,
                                     perf_mode=DR)
                return ps_s, p_t

            def attn_block(b, qb, pre_qk=None, next_unit=None):
                tq0 = b * seq + qb * tb
                ntk = (qb + 1) * dtiles
                ps_yt = [psum.tile([128, dtiles * (hd + 1)], F32,
                                   name=f"ps_yt{h}", tag=f"yt{h}", bufs=1)
                         for h in range(hpc)]
                yt_sb = work.tile([128, dtiles * fl], BF16, name="yt_sb",
                                  tag="ytsb", bufs=4)

                def c0_of(tki):
                    m = tki - qb * dtiles
                    return 128 * m if m > 0 else 0

                cur = pre_qk if pre_qk is not None else qk_scores(b, qb, 0)
                next_qk = None
                for tki in range(ntk):
                    gti = (b * seq) // 128 + tki
                    m = tki - qb * dtiles
                    c0 = c0_of(tki)
                    ps_s, p_t = cur
                    sv = ps_s[:].rearrange("p (h q) -> p h q", h=hpc)[:, :, c0:tb]
                    pv = p_t[:].rearrange("p (h q) -> p h q", h=hpc)[:, :, c0:tb]
                    nc.scalar.activation(pv, sv, AF.Exp, scale=SM_SCALE)
                    if tki + 1 < ntk:
                        cur = qk_scores(b, qb, tki + 1)
                    elif next_unit is not None:
                        next_qk = qk_scores(*next_unit, 0)
                    if m >= 0:
                        # diagonal tri-mask on DVE (a collective occupies the
                        # Pool engine for its whole duration in this cost
                        # model, so nothing latency-critical may queue there)
                        ap = p_t[:].rearrange(
                            "p (h q) -> p h q", h=hpc)[:, :, c0:c0 + 128]
                        nc.vector.tensor_mul(
                            ap, ap,
                            tri[:].unsqueeze(1).broadcast_to((128, hpc, 128)))
                    for h in range(hpc):
                        vm = v_sb[:, gti * vw + h * (hd + 1):
                                  gti * vw + (h + 1) * (hd + 1)]
                        for s in range(dtiles):
                            if s < m:
                                continue
                            nc.tensor.matmul(
                                ps_yt[h][:, s * (hd + 1):(s + 1) * (hd + 1)],
                                p_t[:, h * tb + s * 128:h * tb + (s + 1) * 128],
                                vm,
                                start=(tki == 0 and s == 0),
                                stop=(tki == ntk - 1))
                    drain(3 if ntk <= dtiles else (2 if ntk <= 2 * dtiles else 1))

                last = (b, qb) == (bsz - 1, nqb - 1)
                # normalize token-major: y[s] *= 1/denom
                halves = [(0, dtiles)] if not last else [(0, 2), (2, dtiles)]
                for s0, s1 in halves:
                    for h in range(hpc):
                        yv = ps_yt[h][:].rearrange("p (s c) -> p s c", c=hd + 1)
                        recip = work.tile([128, dtiles], F32, name="recip",
                                          tag="recip", bufs=2)
                        rv = recip[:].rearrange("p (s o) -> p s o", o=1)
                        nc.vector.reciprocal(rv[:, s0:s1], yv[:, s0:s1, hd:hd + 1])
                        ytv = yt_sb[:].rearrange(
                            "p (s f) -> p s f",
                            f=fl)[:, s0:s1, h * hd:(h + 1) * hd]
                        nc.vector.tensor_mul(
                            ytv, yv[:, s0:s1, 0:hd],
                            rv[:, s0:s1].broadcast_to((128, s1 - s0, hd)))
                    # staging: subtile s -> a2a_in[b][j=2qb+s//2][(s%2)*128]
                    av = a2a_in[b][:].rearrange("j (r p) f -> j r p f", r=2)
                    ysrc = yt_sb[:].rearrange("p (s f) -> p s f", f=fl)
                    eng = nc.scalar if last else nc.sync
                    if s1 - s0 == dtiles:
                        # one DMA for all 4 subtiles: dram side [p, (j r), f]
                        eng.dma_start(
                            av[2 * qb:2 * qb + 2].rearrange(
                                "j r p f -> p (j r) f"),
                            ysrc)
                    else:
                        # final unit: one DMA per subtile pair
                        eng.dma_start(
                            av[2 * qb + s0 // 2].rearrange("r p f -> p r f"),
                            ysrc[:, s0:s1, :])
                return next_qk

            def a2a_issue(b):
                nc.gpsimd.collective_compute(
                    "AllToAll", mybir.AluOpType.bypass,
                    replica_groups=[list(range(n_cores))],
                    ins=[a2a_in[b][:]], outs=[a2a_out[b][:]],
                )

            # scheduling floors (scheduler-clock): collective-gated receives
            # and the deferred projections must all sequence AFTER the tail
            # attention units, so they land in (and fill) the last
            # collective's window instead of soaking up mid-stream bubbles.
            cb_end = [0.154, 0.156, 0.158, 0.175]
            proj_floor = [0.159, 0.159, 0.159, 0.176]

            def issue_yloc_dma(pb):
                # transposing receive, floored past C_pb's end so its
                # collective wait is pre-satisfied when it reaches the queue
                # head. Batches 0-2 ride the idle SP tail; the last one goes
                # on ACT (whose queue is empty by then).
                eng = nc.scalar if pb == bsz - 1 else nc.sync
                with tc.tile_wait_until(cb_end[pb]):
                    eng.dma_start_transpose(
                        y_loc[pb][:],
                        a2a_out[pb][:].rearrange("i t f -> (i t) f"))

            def push_proj_chunks(pb):
                wpv = wp_sb[:].rearrange("p (i c) -> p i c", i=n_cores)
                ylv = y_loc[pb][:].rearrange("p (i t) -> p i t", i=n_cores)
                ov = out[:, pb * tsb:(pb + 1) * tsb].rearrange(
                    "(dj p) t -> p dj t", p=128)
                for dj2 in range(d // 256):
                    def p_c(dj2=dj2, pb=pb):
                        for half in range(2):
                            dj = dj2 * 2 + half
                            ps_o = psum.tile([128, tsb], F32, name="ps_o",
                                             tag="mm512", bufs=2)
                            for i in range(n_cores):
                                nc.tensor.matmul(
                                    ps_o[:], wpv[:, i, dj * 128:(dj + 1) * 128],
                                    ylv[:, i, :], start=(i == 0),
                                    stop=(i == n_cores - 1))
                            o_sb = work.tile([128, tsb], F32, name="o_sb",
                                             tag="osb", bufs=6)
                            nc.vector.tensor_copy(o_sb[:], ps_o[:])
                            nc.sync.dma_start(ov[:, dj], o_sb[:])

                    pending_proj.append(p_c)

            units = [(b, qb) for b in range(bsz) for qb in range(nqb)]
            x0 = issue_x_dma(*units[0], split_x=True)
            xts = {1: issue_x_dma(*units[1])}
            push_qkv_chunks(*units[0], x0)
            drain()
            pre_qk = None
            for L, (b, qb) in enumerate(units):
                if L >= 1:
                    drain()
                if L + 1 < len(units):
                    push_qkv_chunks(*units[L + 1], xts.pop(L + 1))
                if L + 2 < len(units):
                    xts[L + 2] = issue_x_dma(*units[L + 2])
                if L == 3:
                    nc.sync.dma_start(
                        wp_sb[:].rearrange("p (i c) -> p i c", i=n_cores),
                        wproj[:, :].rearrange("(i p) c -> p i c", p=128))
                nxt = units[L + 1] if L + 1 < len(units) else None
                pre_qk = attn_block(b, qb, pre_qk=pre_qk, next_unit=nxt)
                if qb == nqb - 1:
                    a2a_issue(b)
                    issue_yloc_dma(b)
                    if b < bsz - 1:
                        push_proj_chunks(b)
            drain()
            # batches 0-2 proj fill the last collective's window, floored so
            # the scheduler cannot hoist them ahead of tail attention units
            # (it does not model collective latency when ordering).
            n_chunks = len(pending_proj)
            per_b = n_chunks // (bsz - 1)
            for ci in range(n_chunks):
                with tc.tile_wait_until(proj_floor[min(ci // per_b, 2)]):
                    pending_proj.pop(0)()
            push_proj_chunks(bsz - 1)
            while pending_proj:
                with tc.tile_wait_until(proj_floor[3]):
                    pending_proj.pop(0)()
    _split_multi_waits(nc)
    return nc


def shard_inputs(x, w_qkv, w_proj, n_cores=N_CORES, n_head=N_HEAD):
    d = x.shape[-1]
    T = x.shape[0] * x.shape[1]
    hpc = n_head // n_cores
    fl = hpc * HD
    kd = d // 128
    npair = kd // 2

    xT = np.ascontiguousarray(np.asarray(x, np.float32).reshape(T, d).T)
    xh = xT.astype(E4NP)
    xl = (xT - xh.astype(np.float32)).astype(E4NP)
    # xhl[j = 2i+hl] = rows i*128..(i+1)*128 of (hl? xl: xh)
    xhl = np.empty((2 * kd, 128, T), E4NP)
    for i in range(kd):
        xhl[2 * i] = xh[i * 128:(i + 1) * 128]
        xhl[2 * i + 1] = xl[i * 128:(i + 1) * 128]

    wq = np.asarray(w_qkv, np.float32)
    wp = np.ascontiguousarray(np.asarray(w_proj, np.float32).T.astype(BF16NP))
    in_maps = []
    for c in range(n_cores):
        r0 = c * fl
        w3 = np.concatenate(
            [wq[r0:r0 + fl], wq[d + r0:d + r0 + fl],
             wq[2 * d + r0:2 * d + r0 + fl]], axis=0).T * 32.0  # [d, 3fl]
        wh = w3.astype(E4NP)
        wl = (w3 - wh.astype(np.float32)).astype(E4NP)
        whl = np.empty((128, npair, 2, 2, 3 * fl), E4NP)
        for jp in range(npair):
            for g in range(2):
                r = (jp * 2 + g) * 128
                whl[:, jp, g, 0] = wh[r:r + 128]
                whl[:, jp, g, 1] = wl[r:r + 128]
        in_maps.append({"xhl": xhl, "whl": whl, "wproj": wp})
    return in_maps


def assemble_out(outs, n_cores=N_CORES, bsz=BSZ, seq=SEQ, d=D):
    tsb = seq // n_cores
    T = bsz * seq
    outT = np.empty((d, T), np.float32)
    for c in range(n_cores):
        for b in range(bsz):
            outT[:, b * seq + c * tsb:b * seq + (c + 1) * tsb] = \
                outs[c][:, b * tsb:(b + 1) * tsb]
    return np.ascontiguousarray(outT.T).reshape(bsz, seq, d)


_NC_CACHE = {}


def kernel(x, w_qkv, w_proj):
    key = "full"
    if key not in _NC_CACHE:
        _NC_CACHE[key] = build_nc()
    nc = _NC_CACHE[key]
    in_maps = shard_inputs(x, w_qkv, w_proj)
    res = run_bass_kernel_spmd(nc, in_maps, list(range(N_CORES))).results
    return assemble_out([res[c]["out"] for c in range(N_CORES)]).astype(np.float32)


# revision 3
# speedup vs baseline: 1.0042x; 1.0042x over previous
"""Causal self-attention (dense transformer) on 8 TRN2 NeuronCores.

Sharding: heads+batch tensor-parallel; each core owns 2 heads for all batches.
Key speedups over the bf16 baseline:
  - QKV projection in compensated fp8 (hi/lo e4m3 splits of x and 32*w prepared
    on the host) with DoubleRow matmuls: 3 passes (hh, hl, lh) at 0.5 cyc/col
    = 0.75x the bf16 cost at bf16-level accuracy.
  - QK scores in fp8 DoubleRow with a zero-padded pair tile: q/k live in one
    SBUF block [q | zeros | k]; the DR pair dim points the second reduction
    tile at the shared zero block, so the 64-deep contraction runs at
    0.5 cyc/col without any partition remapping.
  - Softmax exp on ACT (table pre-warmed, scale=1/sqrt(hd) applied inside
    the activation); diagonal tri-masks on DVE.
  - PV (flipped, ones-column denominator) and output projection stay bf16.
  - Tail: per-batch AllToAll; collective-gated receive DMAs and the deferred
    output projections carry tile_wait_until floors so the tile scheduler
    (which does not model collective latency) cannot sequence them ahead of
    tail attention work; batches 0-2's projections fill the last collective's
    window.

Storage fp32 in/out; PSUM accumulation fp32 everywhere.
"""

import numpy as np
import ml_dtypes

import concourse.bass as bass
import concourse.mybir as mybir
import concourse.tile as tile
from concourse.bass_utils import run_bass_kernel_spmd

BF16 = mybir.dt.bfloat16
F32 = mybir.dt.float32
E4 = mybir.dt.float8e4
AF = mybir.ActivationFunctionType
DR = mybir.MatmulPerfMode.DoubleRow
E4NP = ml_dtypes.float8_e4m3
BF16NP = ml_dtypes.bfloat16

N_CORES = 8
BSZ, SEQ, D, N_HEAD = 4, 2048, 1024, 16
HD = 64

SM_SCALE = float(1.0 / np.sqrt(HD))  # softmax scale, applied inside ACT Exp


def _split_multi_waits(nc):
    """walrus accepts at most ONE sync-wait per instruction: hoist extras onto
    same-engine nops placed immediately before (queue order preserves sems)."""
    edits = []
    for func in nc.m.functions:
        for bb in func.blocks:
            for idx, ins in enumerate(bb.instructions):
                si = ins.sync_info
                if si is not None and len(si.on_wait) > 1:
                    edits.append((bb, idx, ins))
    for bb, idx, ins in reversed(edits):
        si = ins.sync_info
        extra, keep = list(si.on_wait[:-1]), [si.on_wait[-1]]
        ins.sync_info = mybir.SyncInfo(on_wait=keep, on_update=list(si.on_update))
        nops = []
        for w in extra:
            nop = nc.engines[ins.engine].nop().ins
            host = nc.cur_bb.bb.instructions
            assert host[-1] is nop
            host.pop()
            nop.sync_info = mybir.SyncInfo(on_wait=[w], on_update=[])
            nops.append(nop)
        live = bb.instructions
        for j, nop in enumerate(nops):
            live.insert(idx + j, nop)


def build_nc(n_cores=N_CORES, bsz=BSZ, seq=SEQ, d=D, n_head=N_HEAD):
    hd = HD
    hpc = n_head // n_cores          # heads per core
    fl = hpc * hd                    # local feature width
    T = bsz * seq                    # total tokens
    kd = d // 128                    # 128-contraction tiles over d
    npair = kd // 2                  # DoubleRow pairs over d
    tb = 512                         # tq block width
    nqb = seq // tb                  # q-blocks per batch
    dtiles = tb // 128               # 128-token subtiles per q-block
    nt = T // 128                    # total 128-token tiles
    tsb = seq // n_cores             # per-batch token chunk per core (a2a)

    nc = bass.Bass(num_devices=n_cores)
    # x hi/lo fp8 interleaved: tile j = 2*i + hl holds rows i*128..(i+1)*128 of
    # (x hi if hl==0 else x residual)
    xhl = nc.declare_dram_parameter("xhl", [2 * kd, 128, T], E4, isOutput=False)
    # w hi/lo DR layout: [128, jp, g, hl, 3*fl]; contraction row (jp*2+g)*128+p
    whl = nc.declare_dram_parameter("whl", [128, npair, 2, 2, 3 * fl], E4,
                                    isOutput=False)
    wproj = nc.declare_dram_parameter("wproj", [d, d], BF16, isOutput=False)
    out = nc.declare_dram_parameter("out", [d, bsz * tsb], F32, isOutput=True)
    a2a_in = [nc.dram_tensor(f"a2a_in{b}", [n_cores, tsb, fl], BF16)
              for b in range(bsz)]
    a2a_out = [nc.dram_tensor(f"a2a_out{b}", [n_cores, tsb, fl], BF16)
               for b in range(bsz)]

    with tile.TileContext(nc) as tc:
        with (
            tc.tile_pool(name="const", bufs=1) as const,
            tc.tile_pool(name="xin", bufs=2) as xin,
            tc.tile_pool(name="work", bufs=3) as work,
            tc.tile_pool(name="psum", bufs=1, space="PSUM") as psum,
        ):
            # ---- persistent SBUF ----
            w_sb = const.tile([128, npair * 2 * 2 * 3 * fl], E4, name="w_sb")
            wv = w_sb.rearrange("p (j g h c) -> p j g h c", j=npair, g=2, h=2)
            wflat = whl[:].rearrange("p a b c d -> p (a b c d)")
            wseg = 2 * 2 * 3 * fl  # one j-pair's bytes
            nc.scalar.dma_start(w_sb[:, 0:wseg], wflat[:, 0:wseg])
            nc.scalar.dma_start(w_sb[:, wseg:], wflat[:, wseg:])
            wp_sb = const.tile([128, n_cores * d], BF16, name="wp_sb")

            # q | zeros | k fp8 blocks (zero block shared by both DR pairs)
            qkz = const.tile([128, 3 * T], E4, name="qkz")
            nc.gpsimd.memset(qkz[:, T:2 * T], 0.0)

            y_loc = [const.tile([128, n_cores * tsb], BF16, name=f"y_loc{b}")
                     for b in range(bsz)]

            # v token-major bf16 with ones column per (tile, head)
            vw = hpc * (hd + 1)
            v_sb = const.tile([128, nt * vw], BF16, name="v_sb")
            ones_ap = v_sb.rearrange("p (n h c) -> p n h c", h=hpc,
                                     c=hd + 1)[:, :, :, hd:hd + 1]
            nc.vector.memset(ones_ap, 1.0)

            # triangular mask
            tri = const.tile([128, 128], BF16, name="tri")
            nc.gpsimd.memset(tri[:], 1.0)
            nc.gpsimd.affine_select(
                out=tri[:], in_=tri[:],
                compare_op=mybir.AluOpType.is_ge, fill=0.0,
                base=0, channel_multiplier=-1, pattern=[[1, 128]],
            )

            pending = []       # qkv chunks for upcoming units
            pending_proj = []  # output-projection chunks (tail filler)

            def drain(n=None):
                k = len(pending) if n is None else min(n, len(pending))
                for _ in range(k):
                    pending.pop(0)()

            def issue_x_dma(b, qb, split_x=False):
                tbi = b * nqb + qb
                ts0 = tbi * tb
                # [128, 2kd tiles, tb] fp8: tile j at cols j*tb
                x_t = xin.tile([128, 2 * kd * tb], E4, name="x_t", tag="x",
                               bufs=4)
                xsrc = xhl[:, :, ts0:ts0 + tb]
                xdst = x_t[:].rearrange("p (j t) -> p j t", j=2 * kd)
                if split_x:
                    q = (2 * kd) // 4
                    for j in range(4):
                        nc.sync.dma_start(
                            xdst[:, j * q:(j + 1) * q],
                            xsrc[j * q:(j + 1) * q].rearrange("j p t -> p j t"))
                else:
                    nc.sync.dma_start(
                        xdst, xsrc.rearrange("j p t -> p j t"))
                return x_t

            def push_qkv_chunks(b, qb, x_t):
                tbi = b * nqb + qb
                ts0 = tbi * tb
                xv = x_t[:].rearrange("p (j t) -> p j t", j=2 * kd)

                def moving(m, hl, c0, cw):
                    # tiles j = 4m+hl and 4m+2+hl -> [128, 2, cw]
                    ap = xv[:, 4 * m + hl:4 * m + hl + 3:2, c0:c0 + cw]
                    return ap

                def stat(m, hl, c0, cw):
                    # stationary w [128, 2, cw] for pair m, hi/lo hl
                    return wv[:, m, :, hl, c0:c0 + cw]

                # q and k (feature-major [fl, tb]): 3 comp passes x npair DR
                PASSES = ((0, 0), (0, 1), (1, 0))  # (x_hl, w_hl)
                for which, dst0 in ((0, 0), (1, 2 * T)):
                    st = {}

                    def qk_pass(pi, which=which, dst0=dst0, st=st):
                        xh_, wh_ = PASSES[pi]
                        if pi == 0:
                            st["ps"] = psum.tile([fl, tb], F32, name="ps_qk",
                                                 tag="mm512", bufs=2)
                        ps = st["ps"]
                        for m in range(npair):
                            nc.tensor.matmul(
                                ps[:], stat(m, wh_, which * fl, fl),
                                moving(m, xh_, 0, tb),
                                start=(pi == 0 and m == 0),
                                stop=(pi == 2 and m == npair - 1),
                                perf_mode=DR)
                        if pi == 2:
                            # 1/32 undoes the host-side w*32; quantize q/k to
                            # fp8 at natural scale (~N(0,1)) so values stay
                            # clear of e4m3's coarse subnormal range
                            nc.vector.tensor_scalar_mul(
                                qkz[:, dst0 + ts0:dst0 + ts0 + tb], ps[:],
                                1.0 / 32.0)

                    for pi in range(3):
                        pending.append(lambda pi=pi, f=qk_pass: f(pi))

                # v (token-major [128 tok, fl] bf16, scaled 1/32)
                for tt in range(dtiles):
                    vst = {}

                    def v_chunk(tt=tt, vst=vst):
                        gti = tbi * dtiles + tt
                        ps_v = psum.tile([128, fl], F32, name="ps_v",
                                         tag="mm512", bufs=2)
                        for pi in range(3):
                            xh_, wh_ = PASSES[pi]
                            for m in range(npair):
                                nc.tensor.matmul(
                                    ps_v[:],
                                    moving(m, xh_, tt * 128, 128),
                                    stat(m, wh_, 2 * fl, fl),
                                    start=(pi == 0 and m == 0),
                                    stop=(pi == 2 and m == npair - 1),
                                    perf_mode=DR)
                        nc.vector.tensor_scalar_mul(
                            v_sb.rearrange("p (n h c) -> p n h c", h=hpc,
                                           c=hd + 1)[:, gti, :, 0:hd],
                            ps_v[:].rearrange("p (h c) -> p h c", c=hd),
                            1.0 / 32.0)

                    pending.append(v_chunk)

            qz = qkz[:].rearrange("p (z t) -> p z t", z=3)

            def qk_scores(b, qb, tki):
                tq0 = b * seq + qb * tb
                t0k = b * seq + tki * 128
                m = tki - qb * dtiles
                c0 = 128 * m if m > 0 else 0
                ps_s = psum.tile([128, hpc * tb], F32, name="ps_s",
                                 tag="s2", bufs=2)
                p_t = work.tile([128, hpc * tb], BF16, name="p_t",
                                tag="pt", bufs=6)
                for h in range(hpc):
                    hs = slice(h * hd, (h + 1) * hd)
                    # lhsT: [64, 2, 128]: pair g0 = k block (z=2), g1 = zeros
                    lhsT = qz[hs, 1:3, t0k:t0k + 128][:, ::-1]
                    rhs = qz[hs, 0:2, tq0 + c0:tq0 + tb]
                    nc.tensor.matmul(ps_s[:, h * tb + c0:(h + 1) * tb],
                                     lhsT, rhs, start=True, stop=True,
                                     perf_mode=DR)
                return ps_s, p_t

            def attn_block(b, qb, pre_qk=None, next_unit=None):
                tq0 = b * seq + qb * tb
                ntk = (qb + 1) * dtiles
                ps_yt = [psum.tile([128, dtiles * (hd + 1)], F32,
                                   name=f"ps_yt{h}", tag=f"yt{h}", bufs=1)
                         for h in range(hpc)]
                yt_sb = work.tile([128, dtiles * fl], BF16, name="yt_sb",
                                  tag="ytsb", bufs=4)

                def c0_of(tki):
                    m = tki - qb * dtiles
                    return 128 * m if m > 0 else 0

                cur = pre_qk if pre_qk is not None else qk_scores(b, qb, 0)
                next_qk = None
                for tki in range(ntk):
                    gti = (b * seq) // 128 + tki
                    m = tki - qb * dtiles
                    c0 = c0_of(tki)
                    ps_s, p_t = cur
                    sv = ps_s[:].rearrange("p (h q) -> p h q", h=hpc)[:, :, c0:tb]
                    pv = p_t[:].rearrange("p (h q) -> p h q", h=hpc)[:, :, c0:tb]
                    nc.scalar.activation(pv, sv, AF.Exp, scale=SM_SCALE)
                    if tki + 1 < ntk:
                        cur = qk_scores(b, qb, tki + 1)
                    elif next_unit is not None:
                        next_qk = qk_scores(*next_unit, 0)
                    if m >= 0:
                        # diagonal tri-mask on DVE (a collective occupies the
                        # Pool engine for its whole duration in this cost
                        # model, so nothing latency-critical may queue there)
                        ap = p_t[:].rearrange(
                            "p (h q) -> p h q", h=hpc)[:, :, c0:c0 + 128]
                        nc.vector.tensor_mul(
                            ap, ap,
                            tri[:].unsqueeze(1).broadcast_to((128, hpc, 128)))
                    for h in range(hpc):
                        vm = v_sb[:, gti * vw + h * (hd + 1):
                                  gti * vw + (h + 1) * (hd + 1)]
                        for s in range(dtiles):
                            if s < m:
                                continue
                            nc.tensor.matmul(
                                ps_yt[h][:, s * (hd + 1):(s + 1) * (hd + 1)],
                                p_t[:, h * tb + s * 128:h * tb + (s + 1) * 128],
                                vm,
                                start=(tki == 0 and s == 0),
                                stop=(tki == ntk - 1))
                    drain(3 if ntk <= dtiles else (2 if ntk <= 2 * dtiles else 1))

                last = (b, qb) == (bsz - 1, nqb - 1)
                # normalize token-major: y[s] *= 1/denom
                halves = [(0, dtiles)] if not last else [(0, 2), (2, dtiles)]
                for s0, s1 in halves:
                    for h in range(hpc):
                        yv = ps_yt[h][:].rearrange("p (s c) -> p s c", c=hd + 1)
                        recip = work.tile([128, dtiles], F32, name="recip",
                                          tag="recip", bufs=2)
                        rv = recip[:].rearrange("p (s o) -> p s o", o=1)
                        nc.vector.reciprocal(rv[:, s0:s1], yv[:, s0:s1, hd:hd + 1])
                        ytv = yt_sb[:].rearrange(
                            "p (s f) -> p s f",
                            f=fl)[:, s0:s1, h * hd:(h + 1) * hd]
                        nc.vector.tensor_mul(
                            ytv, yv[:, s0:s1, 0:hd],
                            rv[:, s0:s1].broadcast_to((128, s1 - s0, hd)))
                    # staging: subtile s -> a2a_in[b][j=2qb+s//2][(s%2)*128]
                    av = a2a_in[b][:].rearrange("j (r p) f -> j r p f", r=2)
                    ysrc = yt_sb[:].rearrange("p (s f) -> p s f", f=fl)
                    eng = nc.scalar if last else nc.sync
                    if s1 - s0 == dtiles:
                        # one DMA for all 4 subtiles: dram side [p, (j r), f]
                        eng.dma_start(
                            av[2 * qb:2 * qb + 2].rearrange(
                                "j r p f -> p (j r) f"),
                            ysrc)
                    else:
                        # final unit: one DMA per subtile pair
                        eng.dma_start(
                            av[2 * qb + s0 // 2].rearrange("r p f -> p r f"),
                            ysrc[:, s0:s1, :])
                return next_qk

            def a2a_issue(b):
                nc.gpsimd.collective_compute(
                    "AllToAll", mybir.AluOpType.bypass,
                    replica_groups=[list(range(n_cores))],
                    ins=[a2a_in[b][:]], outs=[a2a_out[b][:]],
                )

            # scheduling floors (scheduler-clock): collective-gated receives
            # and the deferred projections must all sequence AFTER the tail
            # attention units, so they land in (and fill) the last
            # collective's window instead of soaking up mid-stream bubbles.
            cb_end = [0.151, 0.153, 0.155, 0.175]
            proj_floor = [0.157, 0.157, 0.157, 0.176]

            def issue_yloc_dma(pb):
                # transposing receive, floored past C_pb's end so its
                # collective wait is pre-satisfied when it reaches the queue
                # head. Batches 0-2 ride the idle SP tail; the last one goes
                # on ACT (whose queue is empty by then).
                eng = nc.scalar if pb == bsz - 1 else nc.sync
                with tc.tile_wait_until(cb_end[pb]):
                    eng.dma_start_transpose(
                        y_loc[pb][:],
                        a2a_out[pb][:].rearrange("i t f -> (i t) f"))

            def push_proj_chunks(pb):
                wpv = wp_sb[:].rearrange("p (i c) -> p i c", i=n_cores)
                ylv = y_loc[pb][:].rearrange("p (i t) -> p i t", i=n_cores)
                ov = out[:, pb * tsb:(pb + 1) * tsb].rearrange(
                    "(dj p) t -> p dj t", p=128)
                for dj2 in range(d // 256):
                    def p_c(dj2=dj2, pb=pb):
                        for half in range(2):
                            dj = dj2 * 2 + half
                            ps_o = psum.tile([128, tsb], F32, name="ps_o",
                                             tag="mm512", bufs=2)
                            for i in range(n_cores):
                                nc.tensor.matmul(
                                    ps_o[:], wpv[:, i, dj * 128:(dj + 1) * 128],
                                    ylv[:, i, :], start=(i == 0),
                                    stop=(i == n_cores - 1))
                            o_sb = work.tile([128, tsb], F32, name="o_sb",
                                             tag="osb", bufs=6)
                            nc.vector.tensor_copy(o_sb[:], ps_o[:])
                            nc.sync.dma_start(ov[:, dj], o_sb[:])

                    pending_proj.append(p_c)

            units = [(b, qb) for b in range(bsz) for qb in range(nqb)]
            x0 = issue_x_dma(*units[0], split_x=True)
            xts = {1: issue_x_dma(*units[1])}
            push_qkv_chunks(*units[0], x0)
            drain()
            pre_qk = None
            for L, (b, qb) in enumerate(units):
                if L >= 1:
                    drain()
                if L + 1 < len(units):
                    push_qkv_chunks(*units[L + 1], xts.pop(L + 1))
                if L + 2 < len(units):
                    xts[L + 2] = issue_x_dma(*units[L + 2])
                if L == 3:
                    nc.sync.dma_start(
                        wp_sb[:].rearrange("p (i c) -> p i c", i=n_cores),
                        wproj[:, :].rearrange("(i p) c -> p i c", p=128))
                nxt = units[L + 1] if L + 1 < len(units) else None
                pre_qk = attn_block(b, qb, pre_qk=pre_qk, next_unit=nxt)
                if qb == nqb - 1:
                    a2a_issue(b)
                    issue_yloc_dma(b)
                    if b < bsz - 1:
                        push_proj_chunks(b)
            drain()
            # batches 0-2 proj fill the last collective's window, floored so
            # the scheduler cannot hoist them ahead of tail attention units
            # (it does not model collective latency when ordering).
            n_chunks = len(pending_proj)
            per_b = n_chunks // (bsz - 1)
            for ci in range(n_chunks):
                with tc.tile_wait_until(proj_floor[min(ci // per_b, 2)]):
                    pending_proj.pop(0)()
            push_proj_chunks(bsz - 1)
            while pending_proj:
                with tc.tile_wait_until(proj_floor[3]):
                    pending_proj.pop(0)()
    _split_multi_waits(nc)
    return nc


def shard_inputs(x, w_qkv, w_proj, n_cores=N_CORES, n_head=N_HEAD):
    d = x.shape[-1]
    T = x.shape[0] * x.shape[1]
    hpc = n_head // n_cores
    fl = hpc * HD
    kd = d // 128
    npair = kd // 2

    xT = np.ascontiguousarray(np.asarray(x, np.float32).reshape(T, d).T)
    xh = xT.astype(E4NP)
    xl = (xT - xh.astype(np.float32)).astype(E4NP)
    # xhl[j = 2i+hl] = rows i*128..(i+1)*128 of (hl? xl: xh)
    xhl = np.empty((2 * kd, 128, T), E4NP)
    for i in range(kd):
        xhl[2 * i] = xh[i * 128:(i + 1) * 128]
        xhl[2 * i + 1] = xl[i * 128:(i + 1) * 128]

    wq = np.asarray(w_qkv, np.float32)
    wp = np.ascontiguousarray(np.asarray(w_proj, np.float32).T.astype(BF16NP))
    in_maps = []
    for c in range(n_cores):
        r0 = c * fl
        w3 = np.concatenate(
            [wq[r0:r0 + fl], wq[d + r0:d + r0 + fl],
             wq[2 * d + r0:2 * d + r0 + fl]], axis=0).T * 32.0  # [d, 3fl]
        wh = w3.astype(E4NP)
        wl = (w3 - wh.astype(np.float32)).astype(E4NP)
        whl = np.empty((128, npair, 2, 2, 3 * fl), E4NP)
        for jp in range(npair):
            for g in range(2):
                r = (jp * 2 + g) * 128
                whl[:, jp, g, 0] = wh[r:r + 128]
                whl[:, jp, g, 1] = wl[r:r + 128]
        in_maps.append({"xhl": xhl, "whl": whl, "wproj": wp})
    return in_maps


def assemble_out(outs, n_cores=N_CORES, bsz=BSZ, seq=SEQ, d=D):
    tsb = seq // n_cores
    T = bsz * seq
    outT = np.empty((d, T), np.float32)
    for c in range(n_cores):
        for b in range(bsz):
            outT[:, b * seq + c * tsb:b * seq + (c + 1) * tsb] = \
                outs[c][:, b * tsb:(b + 1) * tsb]
    return np.ascontiguousarray(outT.T).reshape(bsz, seq, d)


_NC_CACHE = {}


def kernel(x, w_qkv, w_proj):
    key = "full"
    if key not in _NC_CACHE:
        _NC_CACHE[key] = build_nc()
    nc = _NC_CACHE[key]
    in_maps = shard_inputs(x, w_qkv, w_proj)
    res = run_bass_kernel_spmd(nc, in_maps, list(range(N_CORES))).results
    return assemble_out([res[c]["out"] for c in range(N_CORES)]).astype(np.float32)


# revision 4
# speedup vs baseline: 1.0447x; 1.0403x over previous
"""Causal self-attention (dense transformer) on 8 TRN2 NeuronCores.

Sharding: heads+batch tensor-parallel; each core owns 2 heads for all batches.
Key speedups over the bf16 baseline:
  - QKV projection in compensated fp8 (hi/lo e4m3 splits of x and 32*w prepared
    on the host) with DoubleRow matmuls: 3 passes (hh, hl, lh) at 0.5 cyc/col
    = 0.75x the bf16 cost at bf16-level accuracy.
  - QK scores in fp8 DoubleRow with a zero-padded pair tile: q/k live in one
    SBUF block [q | zeros | k]; the DR pair dim points the second reduction
    tile at the shared zero block, so the 64-deep contraction runs at
    0.5 cyc/col without any partition remapping.
  - Softmax exp on ACT (scale=1/sqrt(hd) applied inside the activation);
    diagonal tri-masks on DVE; q/k quantized at natural scale to stay clear
    of e4m3's coarse subnormal range.
  - PV (flipped, ones-column denominator) and output projection stay bf16.
  - Tail: per-batch AllToAll on the Pool queue; collective-gated receive
    DMAs and deferred output projections carry tile_wait_until floors so the
    tile scheduler (which does not model collective latency when ordering)
    cannot sequence them ahead of tail attention work; batches 0-2's
    projections fill the last collective's window.

Storage fp32 in/out; PSUM accumulation fp32 everywhere.
"""

import numpy as np
import ml_dtypes

import concourse.bass as bass
import concourse.mybir as mybir
import concourse.tile as tile
from concourse.bass_utils import run_bass_kernel_spmd

BF16 = mybir.dt.bfloat16
F32 = mybir.dt.float32
E4 = mybir.dt.float8e4
AF = mybir.ActivationFunctionType
DR = mybir.MatmulPerfMode.DoubleRow
E4NP = ml_dtypes.float8_e4m3
BF16NP = ml_dtypes.bfloat16

N_CORES = 8
BSZ, SEQ, D, N_HEAD = 4, 2048, 1024, 16
HD = 64

SM_SCALE = float(1.0 / np.sqrt(HD))  # softmax scale, applied inside ACT Exp


def _split_multi_waits(nc):
    """walrus accepts at most ONE sync-wait per instruction: hoist extras onto
    same-engine nops placed immediately before (queue order preserves sems)."""
    edits = []
    for func in nc.m.functions:
        for bb in func.blocks:
            for idx, ins in enumerate(bb.instructions):
                si = ins.sync_info
                if si is not None and len(si.on_wait) > 1:
                    edits.append((bb, idx, ins))
    for bb, idx, ins in reversed(edits):
        si = ins.sync_info
        extra, keep = list(si.on_wait[:-1]), [si.on_wait[-1]]
        ins.sync_info = mybir.SyncInfo(on_wait=keep, on_update=list(si.on_update))
        nops = []
        for w in extra:
            nop = nc.engines[ins.engine].nop().ins
            host = nc.cur_bb.bb.instructions
            assert host[-1] is nop
            host.pop()
            nop.sync_info = mybir.SyncInfo(on_wait=[w], on_update=[])
            nops.append(nop)
        live = bb.instructions
        for j, nop in enumerate(nops):
            live.insert(idx + j, nop)


def build_nc(n_cores=N_CORES, bsz=BSZ, seq=SEQ, d=D, n_head=N_HEAD):
    hd = HD
    hpc = n_head // n_cores          # heads per core
    fl = hpc * hd                    # local feature width
    T = bsz * seq                    # total tokens
    kd = d // 128                    # 128-contraction tiles over d
    npair = kd // 2                  # DoubleRow pairs over d
    tb = 512                         # tq block width
    nqb = seq // tb                  # q-blocks per batch
    dtiles = tb // 128               # 128-token subtiles per q-block
    nt = T // 128                    # total 128-token tiles
    tsb = seq // n_cores             # per-batch token chunk per core (a2a)

    nc = bass.Bass(num_devices=n_cores)
    # x hi/lo fp8 interleaved: tile j = 2*i + hl holds rows i*128..(i+1)*128 of
    # (x hi if hl==0 else x residual)
    xhl = nc.declare_dram_parameter("xhl", [2 * kd, 128, T], E4, isOutput=False)
    # w hi/lo DR layout: [128, jp, g, hl, 3*fl]; contraction row (jp*2+g)*128+p
    whl = nc.declare_dram_parameter("whl", [128, npair, 2, 2, 3 * fl], E4,
                                    isOutput=False)
    wproj = nc.declare_dram_parameter("wproj", [d, d], BF16, isOutput=False)
    out = nc.declare_dram_parameter("out", [d, bsz * tsb], F32, isOutput=True)
    a2a_in = [nc.dram_tensor(f"a2a_in{b}", [n_cores, tsb, fl], BF16)
              for b in range(bsz)]
    a2a_out = [nc.dram_tensor(f"a2a_out{b}", [n_cores, tsb, fl], BF16)
               for b in range(bsz)]

    with tile.TileContext(nc) as tc:
        with (
            tc.tile_pool(name="const", bufs=1) as const,
            tc.tile_pool(name="xin", bufs=2) as xin,
            tc.tile_pool(name="work", bufs=3) as work,
            tc.tile_pool(name="psum", bufs=1, space="PSUM") as psum,
        ):
            # ---- persistent SBUF ----
            w_sb = const.tile([128, npair * 2 * 2 * 3 * fl], E4, name="w_sb")
            wv = w_sb.rearrange("p (j g h c) -> p j g h c", j=npair, g=2, h=2)
            wflat = whl[:].rearrange("p a b c d -> p (a b c d)")
            wseg = 2 * 2 * 3 * fl  # one j-pair's bytes
            nc.scalar.dma_start(w_sb[:, 0:wseg], wflat[:, 0:wseg])
            nc.scalar.dma_start(w_sb[:, wseg:], wflat[:, wseg:])
            wp_sb = const.tile([128, n_cores * d], BF16, name="wp_sb")

            # q | zeros | k fp8 blocks (zero block shared by both DR pairs)
            qkz = const.tile([128, 3 * T], E4, name="qkz")
            nc.gpsimd.memset(qkz[:, T:2 * T], 0.0)

            y_loc = [const.tile([128, n_cores * tsb], BF16, name=f"y_loc{b}")
                     for b in range(bsz)]

            # v token-major bf16 with ones column per (tile, head)
            vw = hpc * (hd + 1)
            v_sb = const.tile([128, nt * vw], BF16, name="v_sb")
            ones_ap = v_sb.rearrange("p (n h c) -> p n h c", h=hpc,
                                     c=hd + 1)[:, :, :, hd:hd + 1]
            nc.vector.memset(ones_ap, 1.0)

            # triangular mask
            tri = const.tile([128, 128], BF16, name="tri")
            nc.gpsimd.memset(tri[:], 1.0)
            nc.gpsimd.affine_select(
                out=tri[:], in_=tri[:],
                compare_op=mybir.AluOpType.is_ge, fill=0.0,
                base=0, channel_multiplier=-1, pattern=[[1, 128]],
            )

            pending = []       # qkv chunks for upcoming units
            pending_proj = []  # output-projection chunks (tail filler)

            def drain(n=None):
                k = len(pending) if n is None else min(n, len(pending))
                for _ in range(k):
                    pending.pop(0)()

            def issue_x_dma(b, qb, split_x=False):
                tbi = b * nqb + qb
                ts0 = tbi * tb
                # [128, 2kd tiles, tb] fp8: tile j at cols j*tb
                x_t = xin.tile([128, 2 * kd * tb], E4, name="x_t", tag="x",
                               bufs=4)
                xsrc = xhl[:, :, ts0:ts0 + tb]
                xdst = x_t[:].rearrange("p (j t) -> p j t", j=2 * kd)
                if split_x:
                    q = (2 * kd) // 4
                    for j in range(4):
                        nc.sync.dma_start(
                            xdst[:, j * q:(j + 1) * q],
                            xsrc[j * q:(j + 1) * q].rearrange("j p t -> p j t"))
                else:
                    nc.sync.dma_start(
                        xdst, xsrc.rearrange("j p t -> p j t"))
                return x_t

            def push_qkv_chunks(b, qb, x_t):
                tbi = b * nqb + qb
                ts0 = tbi * tb
                xv = x_t[:].rearrange("p (j t) -> p j t", j=2 * kd)

                def moving(m, hl, c0, cw):
                    # tiles j = 4m+hl and 4m+2+hl -> [128, 2, cw]
                    ap = xv[:, 4 * m + hl:4 * m + hl + 3:2, c0:c0 + cw]
                    return ap

                def stat(m, hl, c0, cw):
                    # stationary w [128, 2, cw] for pair m, hi/lo hl
                    return wv[:, m, :, hl, c0:c0 + cw]

                # q and k (feature-major [fl, tb]): 3 comp passes x npair DR
                PASSES = ((0, 0), (0, 1), (1, 0))  # (x_hl, w_hl)
                for which, dst0 in ((0, 0), (1, 2 * T)):
                    st = {}

                    def qk_pass(pi, which=which, dst0=dst0, st=st):
                        xh_, wh_ = PASSES[pi]
                        if pi == 0:
                            st["ps"] = psum.tile([fl, tb], F32, name="ps_qk",
                                                 tag="mm512", bufs=2)
                        ps = st["ps"]
                        for m in range(npair):
                            nc.tensor.matmul(
                                ps[:], stat(m, wh_, which * fl, fl),
                                moving(m, xh_, 0, tb),
                                start=(pi == 0 and m == 0),
                                stop=(pi == 2 and m == npair - 1),
                                perf_mode=DR)
                        if pi == 2:
                            # 1/32 undoes the host-side w*32; quantize q/k to
                            # fp8 at natural scale (~N(0,1)) so values stay
                            # clear of e4m3's coarse subnormal range
                            nc.vector.tensor_scalar_mul(
                                qkz[:, dst0 + ts0:dst0 + ts0 + tb], ps[:],
                                1.0 / 32.0)

                    for pi in range(3):
                        pending.append(lambda pi=pi, f=qk_pass: f(pi))

                # v (token-major [128 tok, fl] bf16, scaled 1/32)
                for tt in range(dtiles):
                    vst = {}

                    def v_chunk(tt=tt, vst=vst):
                        gti = tbi * dtiles + tt
                        ps_v = psum.tile([128, fl], F32, name="ps_v",
                                         tag="mm512", bufs=2)
                        for pi in range(3):
                            xh_, wh_ = PASSES[pi]
                            for m in range(npair):
                                nc.tensor.matmul(
                                    ps_v[:],
                                    moving(m, xh_, tt * 128, 128),
                                    stat(m, wh_, 2 * fl, fl),
                                    start=(pi == 0 and m == 0),
                                    stop=(pi == 2 and m == npair - 1),
                                    perf_mode=DR)
                        nc.vector.tensor_scalar_mul(
                            v_sb.rearrange("p (n h c) -> p n h c", h=hpc,
                                           c=hd + 1)[:, gti, :, 0:hd],
                            ps_v[:].rearrange("p (h c) -> p h c", c=hd),
                            1.0 / 32.0)

                    pending.append(v_chunk)

            qz = qkz[:].rearrange("p (z t) -> p z t", z=3)

            def qk_scores(b, qb, tki):
                tq0 = b * seq + qb * tb
                t0k = b * seq + tki * 128
                m = tki - qb * dtiles
                c0 = 128 * m if m > 0 else 0
                ps_s = psum.tile([128, hpc * tb], F32, name="ps_s",
                                 tag="s2", bufs=2)
                p_t = work.tile([128, hpc * tb], BF16, name="p_t",
                                tag="pt", bufs=6)
                for h in range(hpc):
                    hs = slice(h * hd, (h + 1) * hd)
                    # lhsT: [64, 2, 128]: pair g0 = k block (z=2), g1 = zeros
                    lhsT = qz[hs, 1:3, t0k:t0k + 128][:, ::-1]
                    rhs = qz[hs, 0:2, tq0 + c0:tq0 + tb]
                    nc.tensor.matmul(ps_s[:, h * tb + c0:(h + 1) * tb],
                                     lhsT, rhs, start=True, stop=True,
                                     perf_mode=DR)
                return ps_s, p_t

            def attn_block(b, qb, pre_qk=None, next_unit=None):
                tq0 = b * seq + qb * tb
                ntk = (qb + 1) * dtiles
                ps_yt = [psum.tile([128, dtiles * (hd + 1)], F32,
                                   name=f"ps_yt{h}", tag=f"yt{h}", bufs=1)
                         for h in range(hpc)]
                yt_sb = work.tile([128, dtiles * fl], BF16, name="yt_sb",
                                  tag="ytsb", bufs=4)

                def c0_of(tki):
                    m = tki - qb * dtiles
                    return 128 * m if m > 0 else 0

                cur = pre_qk if pre_qk is not None else qk_scores(b, qb, 0)
                next_qk = None
                for tki in range(ntk):
                    gti = (b * seq) // 128 + tki
                    m = tki - qb * dtiles
                    c0 = c0_of(tki)
                    ps_s, p_t = cur
                    sv = ps_s[:].rearrange("p (h q) -> p h q", h=hpc)[:, :, c0:tb]
                    pv = p_t[:].rearrange("p (h q) -> p h q", h=hpc)[:, :, c0:tb]
                    nc.scalar.activation(pv, sv, AF.Exp, scale=SM_SCALE)
                    if tki + 1 < ntk:
                        cur = qk_scores(b, qb, tki + 1)
                    elif next_unit is not None:
                        next_qk = qk_scores(*next_unit, 0)
                    if m >= 0:
                        # diagonal tri-mask on DVE (a collective occupies the
                        # Pool engine for its whole duration in this cost
                        # model, so nothing latency-critical may queue there)
                        ap = p_t[:].rearrange(
                            "p (h q) -> p h q", h=hpc)[:, :, c0:c0 + 128]
                        nc.vector.tensor_mul(
                            ap, ap,
                            tri[:].unsqueeze(1).broadcast_to((128, hpc, 128)))
                    for h in range(hpc):
                        vm = v_sb[:, gti * vw + h * (hd + 1):
                                  gti * vw + (h + 1) * (hd + 1)]
                        for s in range(dtiles):
                            if s < m:
                                continue
                            nc.tensor.matmul(
                                ps_yt[h][:, s * (hd + 1):(s + 1) * (hd + 1)],
                                p_t[:, h * tb + s * 128:h * tb + (s + 1) * 128],
                                vm,
                                start=(tki == 0 and s == 0),
                                stop=(tki == ntk - 1))
                    drain(3 if ntk <= dtiles else (2 if ntk <= 2 * dtiles else 1))

                last = (b, qb) == (bsz - 1, nqb - 1)
                # normalize token-major: y[s] *= 1/denom
                halves = [(0, dtiles)] if not last else [(0, 2), (2, dtiles)]
                for s0, s1 in halves:
                    for h in range(hpc):
                        yv = ps_yt[h][:].rearrange("p (s c) -> p s c", c=hd + 1)
                        recip = work.tile([128, dtiles], F32, name="recip",
                                          tag="recip", bufs=2)
                        rv = recip[:].rearrange("p (s o) -> p s o", o=1)
                        nc.vector.reciprocal(rv[:, s0:s1], yv[:, s0:s1, hd:hd + 1])
                        ytv = yt_sb[:].rearrange(
                            "p (s f) -> p s f",
                            f=fl)[:, s0:s1, h * hd:(h + 1) * hd]
                        nc.vector.tensor_mul(
                            ytv, yv[:, s0:s1, 0:hd],
                            rv[:, s0:s1].broadcast_to((128, s1 - s0, hd)))
                    # staging: subtile s -> a2a_in[b][j=2qb+s//2][(s%2)*128]
                    av = a2a_in[b][:].rearrange("j (r p) f -> j r p f", r=2)
                    ysrc = yt_sb[:].rearrange("p (s f) -> p s f", f=fl)
                    eng = (nc.sync if s0 == 0 else nc.scalar) if last \
                        else nc.sync
                    if s1 - s0 == dtiles:
                        # one DMA for all 4 subtiles: dram side [p, (j r), f]
                        eng.dma_start(
                            av[2 * qb:2 * qb + 2].rearrange(
                                "j r p f -> p (j r) f"),
                            ysrc)
                    else:
                        # final unit: one DMA per subtile pair
                        eng.dma_start(
                            av[2 * qb + s0 // 2].rearrange("r p f -> p r f"),
                            ysrc[:, s0:s1, :])
                return next_qk

            def a2a_issue(b):
                nc.gpsimd.collective_compute(
                    "AllToAll", mybir.AluOpType.bypass,
                    replica_groups=[list(range(n_cores))],
                    ins=[a2a_in[b][:]], outs=[a2a_out[b][:]],
                )

            # scheduling floors (scheduler-clock): collective-gated receives
            # and the deferred projections must all sequence AFTER the tail
            # attention units, so they land in (and fill) the last
            # collective's window instead of soaking up mid-stream bubbles.
            cb_end = [0.152, 0.154, 0.156, 0.175]
            proj_floor = [0.158, 0.158, 0.158, 0.176]

            def issue_yloc_dma(pb):
                # transposing receive, floored past C_pb's end so its
                # collective wait is pre-satisfied when it reaches the queue
                # head. Batches 0-2 ride the idle SP tail; the last one goes
                # on ACT (whose queue is empty by then).
                eng = nc.scalar if pb == bsz - 1 else nc.sync
                with tc.tile_wait_until(cb_end[pb]):
                    eng.dma_start_transpose(
                        y_loc[pb][:],
                        a2a_out[pb][:].rearrange("i t f -> (i t) f"))

            def push_proj_chunks(pb):
                wpv = wp_sb[:].rearrange("p (i c) -> p i c", i=n_cores)
                ylv = y_loc[pb][:].rearrange("p (i t) -> p i t", i=n_cores)
                ov = out[:, pb * tsb:(pb + 1) * tsb].rearrange(
                    "(dj p) t -> p dj t", p=128)
                for dj2 in range(d // 256):
                    def p_c(dj2=dj2, pb=pb):
                        for half in range(2):
                            dj = dj2 * 2 + half
                            ps_o = psum.tile([128, tsb], F32, name="ps_o",
                                             tag="mm512", bufs=2)
                            for i in range(n_cores):
                                nc.tensor.matmul(
                                    ps_o[:], wpv[:, i, dj * 128:(dj + 1) * 128],
                                    ylv[:, i, :], start=(i == 0),
                                    stop=(i == n_cores - 1))
                            o_sb = work.tile([128, tsb], F32, name="o_sb",
                                             tag="osb", bufs=6)
                            nc.vector.tensor_copy(o_sb[:], ps_o[:])
                            nc.sync.dma_start(ov[:, dj], o_sb[:])

                    pending_proj.append(p_c)

            units = [(b, qb) for b in range(bsz) for qb in range(nqb)]
            x0 = issue_x_dma(*units[0], split_x=True)
            xts = {1: issue_x_dma(*units[1])}
            push_qkv_chunks(*units[0], x0)
            drain()
            pre_qk = None
            for L, (b, qb) in enumerate(units):
                if L >= 1:
                    drain()
                if L + 1 < len(units):
                    push_qkv_chunks(*units[L + 1], xts.pop(L + 1))
                if L + 2 < len(units):
                    xts[L + 2] = issue_x_dma(*units[L + 2])
                if L == 3:
                    nc.sync.dma_start(
                        wp_sb[:].rearrange("p (i c) -> p i c", i=n_cores),
                        wproj[:, :].rearrange("(i p) c -> p i c", p=128))
                nxt = units[L + 1] if L + 1 < len(units) else None
                pre_qk = attn_block(b, qb, pre_qk=pre_qk, next_unit=nxt)
                if qb == nqb - 1:
                    a2a_issue(b)
                    issue_yloc_dma(b)
                    if b < bsz - 1:
                        push_proj_chunks(b)
            drain()
            # batches 0-2 proj fill the last collective's window, floored so
            # the scheduler cannot hoist them ahead of tail attention units
            # (it does not model collective latency when ordering).
            n_chunks = len(pending_proj)
            per_b = n_chunks // (bsz - 1)
            for ci in range(n_chunks):
                with tc.tile_wait_until(proj_floor[min(ci // per_b, 2)]):
                    pending_proj.pop(0)()
            push_proj_chunks(bsz - 1)
            while pending_proj:
                with tc.tile_wait_until(proj_floor[3]):
                    pending_proj.pop(0)()
    _split_multi_waits(nc)
    return nc


def shard_inputs(x, w_qkv, w_proj, n_cores=N_CORES, n_head=N_HEAD):
    d = x.shape[-1]
    T = x.shape[0] * x.shape[1]
    hpc = n_head // n_cores
    fl = hpc * HD
    kd = d // 128
    npair = kd // 2

    xT = np.ascontiguousarray(np.asarray(x, np.float32).reshape(T, d).T)
    xh = xT.astype(E4NP)
    xl = (xT - xh.astype(np.float32)).astype(E4NP)
    # xhl[j = 2i+hl] = rows i*128..(i+1)*128 of (hl? xl: xh)
    xhl = np.empty((2 * kd, 128, T), E4NP)
    for i in range(kd):
        xhl[2 * i] = xh[i * 128:(i + 1) * 128]
        xhl[2 * i + 1] = xl[i * 128:(i + 1) * 128]

    wq = np.asarray(w_qkv, np.float32)
    wp = np.ascontiguousarray(np.asarray(w_proj, np.float32).T.astype(BF16NP))
    in_maps = []
    for c in range(n_cores):
        r0 = c * fl
        w3 = np.concatenate(
            [wq[r0:r0 + fl], wq[d + r0:d + r0 + fl],
             wq[2 * d + r0:2 * d + r0 + fl]], axis=0).T * 32.0  # [d, 3fl]
        wh = w3.astype(E4NP)
        wl = (w3 - wh.astype(np.float32)).astype(E4NP)
        whl = np.empty((128, npair, 2, 2, 3 * fl), E4NP)
        for jp in range(npair):
            for g in range(2):
                r = (jp * 2 + g) * 128
                whl[:, jp, g, 0] = wh[r:r + 128]
                whl[:, jp, g, 1] = wl[r:r + 128]
        in_maps.append({"xhl": xhl, "whl": whl, "wproj": wp})
    return in_maps


def assemble_out(outs, n_cores=N_CORES, bsz=BSZ, seq=SEQ, d=D):
    tsb = seq // n_cores
    T = bsz * seq
    outT = np.empty((d, T), np.float32)
    for c in range(n_cores):
        for b in range(bsz):
            outT[:, b * seq + c * tsb:b * seq + (c + 1) * tsb] = \
                outs[c][:, b * tsb:(b + 1) * tsb]
    return np.ascontiguousarray(outT.T).reshape(bsz, seq, d)


_NC_CACHE = {}


def kernel(x, w_qkv, w_proj):
    key = "full"
    if key not in _NC_CACHE:
        _NC_CACHE[key] = build_nc()
    nc = _NC_CACHE[key]
    in_maps = shard_inputs(x, w_qkv, w_proj)
    res = run_bass_kernel_spmd(nc, in_maps, list(range(N_CORES))).results
    return assemble_out([res[c]["out"] for c in range(N_CORES)]).astype(np.float32)
